# revision 26
# baseline (speedup 1.0000x reference)
"""GCN VGAE encoder (3x GCNConv) on 8 Trainium2 NeuronCores.

Strategy (per spec sharding hint): shard nodes across 8 cores, partition
edges by destination node (host-side, a byproduct of the 1D graph
partitioning), replicate weights, AllGather the projected+scaled node
feature table between layers, gather halo features with dma_gather.

Math: A_hat = D^-1/2 (A+I) D^-1/2 factorizes, so
    gcn(x, W) = dinv * [ (x@W)*dinv + A @ ((x@W)*dinv) ] + b
with dinv = 1/sqrt(deg+1) per node. Per-edge work is pure gather +
scatter-add of 64-float rows; scatter-add is a one-hot matmul on PE
(edges sorted by destination tile, PSUM accumulation per tile).
The two output convs share the adjacency, so W_mu|W_log are fused into
one 64-wide layer-2 table pass.

Performance tuning (measured via in-program repeat slope, which cancels
the ~70 ms axon dispatch latency; device time 5.8 ms -> 3.4 ms):
  - 4 SWDGE queues with gather calls round-robined across them
    (num_swdge_queues=4); the dominant cost is the per-256B-descriptor
    processing of the random-row gathers, and one queue serializes it.
  - edges sorted by source within each (quarter, dst-tile) run: gather
    addresses ascend within a chunk -> better DRAM page locality.
  - gather-tile pools deepened to 6 bufs per quarter so dma_gather
    calls run ahead of the consuming one-hot matmuls.
  - every 3rd one-hot is built on the idle Scalar engine as
    relu(1 - |iota - dstloc|) (exact for integer-valued fp32),
    offloading the Vector engine.
Rejected alternatives (measured): gpsimd ap_gather from an SBUF table
(~2.8 ms/pass, slower than dma_gather's ~2 ms -> ~0.7 ms with 4 queues);
single_packet=True (wedges the device).

Third round (2.2 ms -> 0.68 ms):
  - batched one-hot builds: all ~16 one-hots of a destination tile are
    produced by ONE DVE tensor_tensor (is_equal of iota repeated along
    the free dim vs the tile's contiguous dstloc columns broadcast
    across a 128-wide stride-0 axis).  The previous one-per-chunk
    tensor_scalar chain (~1.5k DVE instructions per pass, each gating
    its matmul) was the hidden serializer: removing it took the kernel
    from 2.4 ms to 0.68 ms, essentially the gather floor.
  - scalar-engine one-hot offload retired (sconehot=0): with bf16 DVE
    one-hots the 2-instruction scalar path was the slower side.

Second round (3.4 ms -> 2.2 ms):
  - the layer-2 table build (relu/transpose/matmul) and the output
    epilogue are emitted per-tile INSIDE the aggregation tile loops
    (agg_pass epilogue=), so their PE/DVE/Scalar work hides under the
    gather windows instead of forming serial phases (3.4 -> 3.1 ms).
  - bf16 message tables: t1s/z2s exported to DRAM as bf16 rows padded
    to 256 B (the dma_gather element granularity, so gather descriptor
    count and bytes are unchanged), one-hots built in bf16 (integer
    values <= 255 are exact), and the scatter matmuls run bf16 x bf16
    at 2x PE throughput with fp32 PSUM accumulation (3.1 -> 2.2 ms,
    rel err 1.6e-03 vs the 2e-2 bar).
"""

import numpy as np

P = 128


def _ceil_div(a, b):
    return -(-a // b)


class _Plan:
    """Host-side edge partitioning shared by all cores (SPMD => one
    common chunk structure = max over cores, padded)."""

    def __init__(self, n, n_cores, cpc, src, dst, max_slice_rows=32000):
        assert n % n_cores == 0
        self.n = n
        self.n_cores = n_cores
        self.cpc = cpc                     # chunks per dma_gather call
        self.S = n // n_cores              # nodes per core
        self.T = _ceil_div(self.S, P)      # dst tiles per core
        self.SPAD = self.T * P
        # quarters: gather index is int16 -> table slice rows <= 32000
        nq = 1
        while _ceil_div(n, nq) > max_slice_rows:
            nq *= 2
        self.NQ = nq
        self.QR = _ceil_div(n, nq)         # rows per table slice

        core = dst // self.S
        drel = dst - core * self.S
        tt = drel // P
        loc = (drel % P).astype(np.float32)
        q = src // self.QR
        qsrc = (src - q * self.QR).astype(np.int16)

        T, NQ = self.T, self.NQ
        key = (core * NQ + q) * T + tt
        counts = np.bincount(key, minlength=n_cores * NQ * T).reshape(
            n_cores, NQ, T
        )
        # common run length per (quarter, tile): max over cores, runs are
        # packed back-to-back in the quarter stream (no 128-padding per
        # run; chunks may span adjacent tiles).
        self.rl = counts.max(axis=0)                         # [NQ, T]
        self.run_start = np.zeros((NQ, T), np.int64)
        self.run_start[:, 1:] = np.cumsum(self.rl, axis=1)[:, :-1]
        self.NQE = self.rl.sum(axis=1)                       # edges/quarter
        self.NQC = _ceil_div(self.NQE, P)                    # chunks/quarter
        self.NCH = int(self.NQC.sum())

        # order edges by (core, quarter, tile); rank within group.
        # optional src-ascending order within each run improves DMA
        # page locality of the gathers.
        import os as _osp
        if int(_osp.environ.get("GCN_SRCSORT", "1")):
            sidx = np.lexsort((src, tt, q, core))
        else:
            sidx = np.lexsort((tt, q, core))
        self.sc = core[sidx]
        self.sq = q[sidx]
        self.st = tt[sidx]
        self.sqsrc = qsrc[sidx]
        self.sloc = loc[sidx]
        gkey = (self.sc * NQ + self.sq) * T + self.st
        first = np.r_[True, gkey[1:] != gkey[:-1]]
        gstart = np.flatnonzero(first)
        glen = np.diff(np.r_[gstart, len(gkey)])
        self.rank = np.arange(len(gkey)) - np.repeat(gstart, glen)

        # chunk-part (cp) map: device consumes tiles in order; for tile t
        # and quarter q, the run covers chunks j0..j1 of quarter q's
        # stream; each (t, q, j) overlap gets its own dstloc column.
        self.tile_ops = []        # [T] -> list of (q, j, cp_col)
        self.cp_of = {}           # (q, j, t) -> cp column
        ncp = 0
        for t in range(T):
            ops = []
            for qq in range(NQ):
                r0 = int(self.run_start[qq, t])
                r1 = r0 + int(self.rl[qq, t])
                if r1 == r0:
                    continue
                for j in range(r0 // P, (r1 - 1) // P + 1):
                    ops.append((qq, j, ncp))
                    self.cp_of[(qq, j, t)] = ncp
                    ncp += 1
            self.tile_ops.append(ops)
        self.NCP = ncp
        # vectorized cp lookup: cp = cp_base[t] + ops_before[q,t] + (j - j0)
        self.cp_base = np.zeros(T, np.int64)
        run2 = 0
        self.ops_before = np.zeros((NQ, T), np.int64)
        self.j0 = self.run_start // P
        for t in range(T):
            self.cp_base[t] = run2
            acc = 0
            for qq in range(NQ):
                self.ops_before[qq, t] = acc
                if self.rl[qq, t] > 0:
                    r0 = int(self.run_start[qq, t])
                    r1 = r0 + int(self.rl[qq, t])
                    acc += (r1 - 1) // P - r0 // P + 1
            run2 += acc
        assert run2 == ncp

        # gather calls per quarter
        self.ncalls = [_ceil_div(int(c), cpc) for c in self.NQC]
        # idx tensor column offset of each (quarter, call)
        self.call_col0 = {}
        col = 0
        for qq in range(NQ):
            for k in range(self.ncalls[qq]):
                L = min(cpc, int(self.NQC[qq]) - k * cpc)
                self.call_col0[(qq, k)] = (col, L)
                col += L * 8
        self.IDXCOLS = col

    def core_arrays(self, c, dst, n):
        """Per-core upload tensors: gather idx [128, IDXCOLS] i16,
        dstloc [128, NCP] f32, deg cols [128, T] f32."""
        NQ, T, cpc = self.NQ, self.T, self.cpc
        m_core = self.sc == c
        idx_out = np.zeros((P, self.IDXCOLS), np.int16)
        dl = np.full((self.NCP, P), 255.0, np.float32)
        mloc = self.sloc[m_core]
        mq = self.sq[m_core]
        mt = self.st[m_core]
        mrank = self.rank[m_core]
        msrc = self.sqsrc[m_core]
        # stream position of each edge within its quarter
        pos = self.run_start[mq, mt] + mrank
        cpcol = (self.cp_base[mt] + self.ops_before[mq, mt]
                 + pos // P - self.j0[mq, mt])
        dl[cpcol, pos % P] = mloc
        for qq in range(NQ):
            mm = mq == qq
            arr = np.zeros(int(self.NQC[qq]) * P, np.int16)
            arr[pos[mm]] = msrc[mm]
            for k in range(self.ncalls[qq]):
                c0, L = self.call_col0[(qq, k)]
                seg = arr[k * cpc * P:(k * cpc + L) * P]
                wrapped = seg.reshape(L * 8, 16).T       # [16, L*8]
                idx_out[:, c0:c0 + L * 8] = np.tile(wrapped, (8, 1))
        deg = np.bincount(dst, minlength=n)[c * self.S:(c + 1) * self.S]
        degp = np.zeros(self.SPAD, np.float32)
        degp[:self.S] = deg
        return idx_out, dl.T.copy(), degp.reshape(self.T, P).T.copy()


def _build(plan, d_in, d_h, d_o):
    """Build the SPMD Bass program (same for every core)."""
    import concourse.mybir as mybir
    import concourse.tile as tile
    from concourse import bacc
    from concourse.masks import make_identity

    F32 = mybir.dt.float32
    I16 = mybir.dt.int16
    BF16 = mybir.dt.bfloat16
    n, T, NQ, SPAD, S, QR = plan.n, plan.T, plan.NQ, plan.SPAD, plan.S, plan.QR
    NCH, cpc = plan.NCH, plan.cpc
    n_cores = plan.n_cores

    import os as _os0
    nswq = int(_os0.environ.get("GCN_NSWQ", "4"))
    gbufs = int(_os0.environ.get("GCN_GBUFS", "6"))
    sp = bool(int(_os0.environ.get("GCN_SP", "0")))
    aggbufs = int(_os0.environ.get("GCN_AGGBUFS", "4"))
    mmbufs = int(_os0.environ.get("GCN_MMBUFS", "4"))
    sconehot = int(_os0.environ.get("GCN_SCONEHOT", "0"))
    bf16 = int(_os0.environ.get("GCN_BF16", "1"))
    batchoh = int(_os0.environ.get("GCN_BATCHOH", "1"))
    if batchoh and gbufs > 5:
        gbufs = 5  # make room for the per-tile one-hot tiles
    nc = bacc.Bacc("TRN2", target_bir_lowering=False,
                   debug=False, num_devices=n_cores,
                   num_swdge_queues=nswq)

    x_d = nc.dram_tensor("x", [SPAD, d_in], F32, kind="ExternalInput")
    w1_d = nc.dram_tensor("w1", [d_in, d_h], F32, kind="ExternalInput")
    wc_d = nc.dram_tensor("wcat", [d_h, d_o], F32, kind="ExternalInput")
    b1_d = nc.dram_tensor("b1", [d_h], F32, kind="ExternalInput")
    bc_d = nc.dram_tensor("bcat", [d_o], F32, kind="ExternalInput")
    deg_d = nc.dram_tensor("deg", [P, T], F32, kind="ExternalInput")
    dl_d = nc.dram_tensor("dstloc", [P, plan.NCP], F32, kind="ExternalInput")
    idx_d = nc.dram_tensor("gidx", [P, plan.IDXCOLS], I16, kind="ExternalInput")
    out_d = nc.dram_tensor("out2", [SPAD, d_o], F32, kind="ExternalOutput")

    assert d_h == d_o
    TDT = BF16 if bf16 else F32
    tw = 128 if bf16 else d_h   # table row padded to 256B when bf16
    t1s_own = nc.dram_tensor("t1s_own", [S, tw], TDT, kind="Internal")
    t1s_full = nc.dram_tensor("t1s_full", [n, tw], TDT, kind="Internal",
                              addr_space="Shared")
    z2s_own = nc.dram_tensor("z2s_own", [S, tw], TDT, kind="Internal")
    z2s_full = nc.dram_tensor("z2s_full", [n, tw], TDT, kind="Internal",
                              addr_space="Shared")
    rg = [list(range(n_cores))]

    from contextlib import ExitStack

    with tile.TileContext(nc, num_cores=n_cores) as tc, ExitStack() as st:
        cp = st.enter_context(tc.tile_pool(name="consts", bufs=1))
        bigp = st.enter_context(tc.tile_pool(name="big", bufs=1))
        xp = st.enter_context(tc.tile_pool(name="x", bufs=3))
        xtp = st.enter_context(tc.tile_pool(name="xt", bufs=2))
        expp = st.enter_context(tc.tile_pool(name="exp", bufs=3))
        htp = st.enter_context(tc.tile_pool(name="ht", bufs=2))
        ohp = st.enter_context(tc.tile_pool(name="oh", bufs=8))
        ohtp = st.enter_context(tc.tile_pool(name="oht", bufs=3))
        gps = [st.enter_context(tc.tile_pool(name=f"g{q}", bufs=gbufs))
               for q in range(NQ)]
        mmp = st.enter_context(tc.tile_pool(name="mm", bufs=mmbufs,
                                            space="PSUM"))
        aggp = st.enter_context(tc.tile_pool(name="agg", bufs=aggbufs,
                                             space="PSUM"))

        # ---- constants ----
        iota_i = cp.tile([P, P], mybir.dt.int32)
        nc.gpsimd.iota(iota_i[:], pattern=[[1, P]], base=0,
                       channel_multiplier=0)
        iota_f = cp.tile([P, P], F32)
        nc.vector.tensor_copy(iota_f[:], iota_i[:])
        ident = cp.tile([P, P], F32)
        make_identity(nc, ident[:])
        ones_row = cp.tile([1, P], F32)
        nc.gpsimd.memset(ones_row[:], 1.0)

        w1_sb = cp.tile([d_in, d_h], F32)
        nc.sync.dma_start(w1_sb[:], w1_d[:, :])
        wc_sb = cp.tile([d_h, d_o], F32)
        nc.sync.dma_start(wc_sb[:], wc_d[:, :])
        b1r = cp.tile([1, d_h], F32)
        nc.sync.dma_start(b1r[:], b1_d[None, :])
        bcr = cp.tile([1, d_o], F32)
        nc.sync.dma_start(bcr[:], bc_d[None, :])

        # bias rows broadcast to 128 partitions via ones-matmul
        b1bc = cp.tile([P, d_h], F32)
        ps = mmp.tile([P, P], F32, space="PSUM", tag="mm")
        nc.tensor.matmul(ps[:, :d_h], lhsT=ones_row[:], rhs=b1r[:],
                         start=True, stop=True)
        nc.vector.tensor_copy(b1bc[:], ps[:, :d_h])
        bcbc = cp.tile([P, d_o], F32)
        ps = mmp.tile([P, P], F32, space="PSUM", tag="mm")
        nc.tensor.matmul(ps[:, :d_o], lhsT=ones_row[:], rhs=bcr[:],
                         start=True, stop=True)
        nc.vector.tensor_copy(bcbc[:], ps[:, :d_o])

        deg_sb = cp.tile([P, T], F32)
        nc.sync.dma_start(deg_sb[:], deg_d[:, :])
        sq_sb = cp.tile([P, T], F32)
        # sqrt(deg + 1): +1 is the self-loop
        nc.scalar.activation(sq_sb[:], deg_sb[:],
                             mybir.ActivationFunctionType.Sqrt,
                             bias=1.0, scale=1.0)
        dinv = cp.tile([P, T], F32)
        nc.vector.reciprocal(dinv[:], sq_sb[:])

        dl_sb = cp.tile([P, plan.NCP], F32)
        nc.sync.dma_start(dl_sb[:], dl_d[:, :])
        if bf16:
            # integer-valued (<=255) so bf16 is exact
            iota_b = cp.tile([P, P], BF16)
            nc.vector.tensor_copy(iota_b[:], iota_f[:])
            dlneg_b = cp.tile([P, plan.NCP], BF16)
            nc.vector.tensor_scalar(dlneg_b[:], dl_sb[:], -1.0, None,
                                    mybir.AluOpType.mult)
        else:
            iota_b = iota_f
            dlneg_b = cp.tile([P, plan.NCP], F32)
            nc.vector.tensor_scalar(dlneg_b[:], dl_sb[:], -1.0, None,
                                    mybir.AluOpType.mult)
        TDT0 = BF16 if bf16 else F32
        MAXOPS = max((len(o) for o in plan.tile_ops), default=1)
        if batchoh:
            # iota repeated MAXOPS times along free dim; dl in table dtype
            iota_rep = cp.tile([P, MAXOPS, P], TDT0)
            for m in range(MAXOPS):
                nc.vector.tensor_copy(iota_rep[:, m, :], iota_b[:])
            dl_bb = cp.tile([P, plan.NCP], TDT0)
            nc.vector.tensor_copy(dl_bb[:], dl_sb[:])
        idx_sb = cp.tile([P, plan.IDXCOLS], I16)
        nc.sync.dma_start(idx_sb[:], idx_d[:, :])

        t1s_sb = bigp.tile([P, T, d_h], F32)
        z2s_sb = bigp.tile([P, T, d_o], F32)

        import os as _os2
        _nocoll = bool(_os2.environ.get("GCN_NOCOLL"))
        _repeat = int(_os2.environ.get("GCN_REPEAT", "1"))

        def proj_pass():
            # ---- layer-1 projection: t1s = (x @ W1) * dinv ----
            for t in range(T):
                xt = xp.tile([P, d_in], F32)
                nc.sync.dma_start(xt[:], x_d[t * P:(t + 1) * P, :])
                pst = mmp.tile([P, P], F32, space="PSUM", tag="mm")
                nc.tensor.transpose(pst[:d_in, :], xt[:], ident[:])
                xT = xtp.tile([d_in, P], F32)
                nc.vector.tensor_copy(xT[:], pst[:d_in, :])
                psm = mmp.tile([P, P], F32, space="PSUM", tag="mm")
                nc.tensor.matmul(psm[:, :d_h], lhsT=xT[:], rhs=w1_sb[:],
                                 start=True, stop=True)
                nc.vector.tensor_scalar(t1s_sb[:, t, :], psm[:, :d_h],
                                        dinv[:, t:t + 1], None,
                                        mybir.AluOpType.mult)
                r0 = t * P
                r1 = min(S, r0 + P)
                if r1 > r0:
                    if bf16:
                        ex = expp.tile([P, d_h], BF16, tag="ex")
                        nc.vector.tensor_scalar(ex[:], psm[:, :d_h],
                                                dinv[:, t:t + 1], None,
                                                mybir.AluOpType.mult)
                        nc.sync.dma_start(t1s_own[r0:r1, :d_h],
                                          ex[:r1 - r0, :])
                    else:
                        nc.sync.dma_start(t1s_own[r0:r1, :],
                                          t1s_sb[:r1 - r0, t, :])

            if _nocoll:
                nc.sync.dma_start(t1s_full[0:S, :], t1s_own[:, :])
            else:
                nc.gpsimd.collective_compute(
                    "AllGather", mybir.AluOpType.bypass, replica_groups=rg,
                    ins=[t1s_own[:, :].opt()], outs=[t1s_full[:, :].opt()])

        import os as _os3
        _aggmode = _os3.environ.get("GCN_AGGMODE", "full")

        def agg_pass(table, acc_sb, d_f, epilogue=None):
            """acc_sb[:, t, :] += sum_e onehot(dst) * table[src].

            epilogue(t) is emitted right after tile t\'s accumulation so
            its PE/DVE/Scalar work interleaves with the next tiles\'
            gather windows instead of forming a serial phase."""
            issued = {}
            call_no = [0]
            dummies = {}

            def get_call(qq, k):
                if _aggmode == "compute":
                    if qq not in dummies:
                        g = gps[qq].tile([P, cpc, tw], TDT, tag=f"gt{qq}")
                        nc.gpsimd.memset(g[:], 0.25)
                        dummies[qq] = g
                    return dummies[qq]
                if (qq, k) not in issued:
                    c0, L = plan.call_col0[(qq, k)]
                    g = gps[qq].tile([P, cpc, tw], TDT, tag=f"gt{qq}")
                    q0 = qq * QR
                    q1 = min(n, q0 + QR)
                    nc.gpsimd.dma_gather(
                        out_ap=g[:, :L, :],
                        in_ap=table[q0:q1, :],
                        idxs_ap=idx_sb[:, c0:c0 + L * 8],
                        num_idxs=L * P,
                        num_idxs_reg=L * P,
                        elem_size=tw,
                        single_packet=sp,
                        queue_num=call_no[0] % nswq,
                    )
                    call_no[0] += 1
                    issued[(qq, k)] = g
                return issued[(qq, k)]

            if _aggmode == "gather":
                for qq in range(NQ):
                    for k in range(plan.ncalls[qq]):
                        get_call(qq, k)
                if epilogue is not None:
                    for t in range(T):
                        epilogue(t)
                return

            for t in range(T):
                ops = plan.tile_ops[t]
                if ops:
                    psa = aggp.tile([P, d_f], F32, space="PSUM", tag="agg")
                    oht = None
                    if batchoh:
                        # all of tile t's one-hots in one DVE op: the
                        # cp columns of a tile are contiguous by
                        # construction, broadcast each dl column across
                        # a 128-wide stride-0 axis against repeated iota
                        nops = len(ops)
                        cp0 = ops[0][2]
                        oht = ohtp.tile([P, MAXOPS, P], TDT, tag="oht")
                        nc.vector.tensor_tensor(
                            oht[:, :nops, :], iota_rep[:, :nops, :],
                            dl_bb[:, cp0:cp0 + nops, None].broadcast_to(
                                (P, nops, P)),
                            mybir.AluOpType.is_equal)
                    for i, (qq, j, cpcol) in enumerate(ops):
                        g = get_call(qq, j // cpc)
                        col = j % cpc
                        if oht is not None:
                            nc.tensor.matmul(psa[:], lhsT=oht[:, i, :],
                                             rhs=g[:, col, :d_f],
                                             start=(i == 0),
                                             stop=(i == len(ops) - 1))
                            continue
                        oh = ohp.tile([P, P], TDT, tag="oh")
                        if sconehot and i % sconehot == sconehot - 1:
                            # scalar engine: oh = relu(1 - |iota - dl|)
                            tmp = ohp.tile([P, P], TDT, tag="ohs")
                            nc.scalar.activation(
                                tmp[:], iota_b[:],
                                mybir.ActivationFunctionType.Abs,
                                bias=dlneg_b[:, cpcol:cpcol + 1])
                            nc.scalar.activation(
                                oh[:], tmp[:],
                                mybir.ActivationFunctionType.Relu,
                                bias=1.0, scale=-1.0)
                        else:
                            nc.vector.tensor_scalar(
                                oh[:], iota_b[:], dl_sb[:, cpcol:cpcol + 1],
                                None, mybir.AluOpType.is_equal)
                        nc.tensor.matmul(psa[:], lhsT=oh[:],
                                         rhs=g[:, col, :d_f],
                                         start=(i == 0),
                                         stop=(i == len(ops) - 1))
                    nc.vector.tensor_tensor(acc_sb[:, t, :],
                                            acc_sb[:, t, :],
                                            psa[:], mybir.AluOpType.add)
                if epilogue is not None:
                    epilogue(t)

        import os as _os
        _stage = int(_os.environ.get("GCN_STAGE", "2"))

        def mid_tile(t):
            # ---- h = relu(agg1*dinv + b1); z2s = (h @ Wcat) * dinv ----
            nc.vector.scalar_tensor_tensor(
                t1s_sb[:, t, :], t1s_sb[:, t, :], dinv[:, t:t + 1],
                b1bc[:], mybir.AluOpType.mult, mybir.AluOpType.add)
            nc.scalar.activation(t1s_sb[:, t, :], t1s_sb[:, t, :],
                                 mybir.ActivationFunctionType.Relu)
            pst = mmp.tile([P, P], F32, space="PSUM", tag="mm")
            nc.tensor.transpose(pst[:d_h, :], t1s_sb[:, t, :], ident[:])
            hT = htp.tile([d_h, P], F32)
            nc.vector.tensor_copy(hT[:], pst[:d_h, :])
            psm = mmp.tile([P, P], F32, space="PSUM", tag="mm")
            nc.tensor.matmul(psm[:, :d_o], lhsT=hT[:], rhs=wc_sb[:],
                             start=True, stop=True)
            nc.vector.tensor_scalar(z2s_sb[:, t, :], psm[:, :d_o],
                                    dinv[:, t:t + 1], None,
                                    mybir.AluOpType.mult)
            r0 = t * P
            r1 = min(S, r0 + P)
            if r1 > r0:
                if bf16:
                    ex = expp.tile([P, d_o], BF16, tag="ex2")
                    nc.vector.tensor_scalar(ex[:], psm[:, :d_o],
                                            dinv[:, t:t + 1], None,
                                            mybir.AluOpType.mult)
                    nc.sync.dma_start(z2s_own[r0:r1, :d_o],
                                      ex[:r1 - r0, :])
                else:
                    nc.sync.dma_start(z2s_own[r0:r1, :],
                                      z2s_sb[:r1 - r0, t, :])

        def coll2():
            if _nocoll:
                nc.sync.dma_start(z2s_full[0:S, :], z2s_own[:, :])
            else:
                nc.gpsimd.collective_compute(
                    "AllGather", mybir.AluOpType.bypass, replica_groups=rg,
                    ins=[z2s_own[:, :].opt()], outs=[z2s_full[:, :].opt()])

        def out_tile(t):
            # ---- out2 = agg2 * dinv + bcat ----
            nc.vector.scalar_tensor_tensor(
                z2s_sb[:, t, :], z2s_sb[:, t, :], dinv[:, t:t + 1],
                bcbc[:], mybir.AluOpType.mult, mybir.AluOpType.add)
            nc.sync.dma_start(out_d[t * P:(t + 1) * P, :],
                              z2s_sb[:, t, :])

        for _rep in range(_repeat):
            proj_pass()
            if _stage >= 1:
                agg_pass(t1s_full, t1s_sb, d_h, epilogue=mid_tile)
            else:
                for t in range(T):
                    mid_tile(t)
            coll2()
            if _stage >= 2:
                agg_pass(z2s_full, z2s_sb, d_o, epilogue=out_tile)
            else:
                for t in range(T):
                    out_tile(t)

    nc.compile()
    return nc


_CACHE = {}


def _get_program(n, e, d_in, d_h, d_o, n_cores, cpc, edge_key, src, dst,
                 max_slice_rows=32000):
    key = (n, e, d_in, d_h, d_o, n_cores, cpc, edge_key, max_slice_rows)
    if key not in _CACHE:
        plan = _Plan(n, n_cores, cpc, src, dst, max_slice_rows)
        nc = _build(plan, d_in, d_h, d_o)
        _CACHE[key] = (plan, nc)
    return _CACHE[key]


def kernel(x, edge_index, W1, b1, W_mu, b_mu, W_log, b_log,
           n_cores=8, cpc=16, max_slice_rows=32000, _run_kwargs=None):
    from concourse.bass_utils import run_bass_kernel_spmd

    x = np.asarray(x, np.float32)
    edge_index = np.asarray(edge_index)
    W1 = np.asarray(W1, np.float32)
    Wcat = np.concatenate([np.asarray(W_mu, np.float32),
                           np.asarray(W_log, np.float32)], axis=1)
    bcat = np.concatenate([np.asarray(b_mu, np.float32),
                           np.asarray(b_log, np.float32)])
    b1 = np.asarray(b1, np.float32)
    n, d_in = x.shape
    d_h = W1.shape[1]
    d_o = Wcat.shape[1]
    lat = np.asarray(W_mu, np.float32).shape[1]
    src = edge_index[0].astype(np.int64)
    dst = edge_index[1].astype(np.int64)

    edge_key = hash((src.tobytes(), dst.tobytes()))
    plan, nc = _get_program(n, len(src), d_in, d_h, d_o, n_cores, cpc,
                            edge_key, src, dst, max_slice_rows)

    in_maps = []
    for c in range(n_cores):
        idx_u, dl, deg = plan.core_arrays(c, dst, n)
        xs = np.zeros((plan.SPAD, d_in), np.float32)
        xs[:plan.S] = x[c * plan.S:(c + 1) * plan.S]
        in_maps.append({
            "x": xs, "w1": W1, "wcat": Wcat, "b1": b1, "bcat": bcat,
            "deg": deg, "dstloc": dl, "gidx": idx_u,
        })

    global _LAST_RESULT, _LAST_IN_MAPS, _LAST_DIMS
    _LAST_IN_MAPS = in_maps
    _LAST_DIMS = (d_in, d_h, d_o)
    res = run_bass_kernel_spmd(nc, in_maps, core_ids=list(range(n_cores)),
                               **(_run_kwargs or {}))
    _LAST_RESULT = res
    out = np.concatenate(
        [res.results[c]["out2"][:plan.S] for c in range(n_cores)], axis=0)
    return (out[:, :lat].copy(), out[:, lat:].copy())


_LAST_RESULT = None
_LAST_IN_MAPS = None
_LAST_DIMS = (128, 64, 64)

